# revision 11
# baseline (speedup 1.0000x reference)
"""Trainium2 Bass kernel for nn_Attention_48601849922045.

GQA attention layer (B=2, S=2048, D=2048, H=32 q-heads, KV=8 kv-heads, HD=64)
with llama RoPE, causal softmax, and output projection.

Sharding: tensor-parallel over heads across 8 cores - each core owns one KV
group (1 kv head + its 4 q heads).  x is replicated; per core:

  1. QKV projection, weights-stationary, two passes over the resident x
     chunks (e-tiles {q0,q1} then {k|v}) so it needs only 2 PSUM banks.
  2. RoPE applied in [e, n] layout: partition pair-swap via two strided
     SBUF->SBUF DMAs, then 3 large DVE ops against host-built cos/sin
     tables.  v (no rope) is PE-transposed to natural [t, hd] layout and
     augmented with a ones column so the PV matmul also produces the
     softmax denominator.
  3. Attention per (b, head-PAIR): the local q heads are processed two at
     a time using PE row tiling - kr keeps k duplicated on partitions
     0-63 and 64-127, and qr keeps the pair's heads on the two halves, so
     the even head's scores matmul runs on row tile (0,0) while the odd
     head's runs concurrently on row tile (64,0).  This halves the scores
     phase (K=64 would otherwise idle half the PE array).  Both heads'
     score strips land in one [P, 2, JW] PSUM tile (double-buffered),
     giving a single exp per strip; the exp is sliced to start at the
     strip's causal column, skipping the fully-masked region.  PV is
     K=128 (full array), software-pipelined one strip behind scores.
  4. Eight small per-(b,h) bf16 AllToAlls reshard o from head-sharded to
     row-sharded.  The reshard staging tile orT is SPLIT per head pair:
     a single tile would make every outproj matmul wait on the LAST
     collective (dependencies are tracked per tile), killing the tail.
  5. Row-parallel output projection with the full wo resident in SBUF
     (loaded once), micro-interleaved into the attention phases via
     filler generators (~1us PE quanta) so the PE keeps streaming while
     exp paces the attention pipeline; the final head-pair waves run
     during the last AllToAlls' flight.

Scheduling notes (hard-won): every engine queue executes in order, so a
filler matmul whose inputs aren't ready stalls every attention matmul
behind it (hence start_delay and the orT split); the exp chain on ACT is
the attention pipeline clock, so nothing else may sit on the ACT queue
mid-kernel; both collectives of a pair are issued before either's orT
staging DMAs so the second doesn't queue behind the first's wait.

Host side only shards/transposes inputs and concatenates the 8 output
row-shards.
"""

import os

import numpy as np
import ml_dtypes

import concourse.bass as bass
import concourse.bacc as bacc
import concourse.tile as tile
import concourse.mybir as mybir
from concourse.bass_utils import run_bass_kernel_spmd

P = 128
B, S, D = 2, 2048, 2048
H, KV, HD = 32, 8, 64
NCORES = 8
HL = H // NCORES          # 4 local q heads
BS = B * S                # 4096 rows
EQ, EK, EV = HL * HD, HD, HD
E3 = EQ + EK + EV         # 384 = 3 PE tiles of 128
ET = E3 // P              # 3 e-tiles (0,1: q heads, 2: k|v stacked)
CH = D // P               # 16 contraction chunks
STC = S // P              # 16 t-chunks per batch
NBW = 512                 # qkv n-block width
NBB = S // NBW            # 4 n-blocks per batch
JW = 512                  # attention n-block width
JB = S // JW
RSH = BS // NCORES // B   # 256 rows per (core, batch)
VAW = P                   # v-aug stride: ones at col 64, zero-padded
                          # to 128 cols so the PV stationary gets FWL

F32 = mybir.dt.float32
BF16 = mybir.dt.bfloat16

_CACHE = {}


def _build_nc():
    nc = bacc.Bacc("TRN2", target_bir_lowering=False, debug=False,
                   num_devices=NCORES)

    # xT and woT arrive pre-tiled so every SBUF tile is one contiguous
    # 128KB DRAM block (1KB-row strided loads only reach ~50GB/s)
    xT = nc.dram_tensor("xT", [CH, B * NBB, P, NBW], BF16,
                        kind="ExternalInput").ap()
    wT = nc.dram_tensor("wT", [D, E3], BF16, kind="ExternalInput").ap()
    woT = nc.dram_tensor("woT", [CH, D // JW, P, JW], BF16,
                         kind="ExternalInput").ap()
    cosT = nc.dram_tensor("cosT", [P, S], BF16, kind="ExternalInput").ap()
    sinPM = nc.dram_tensor("sinPM", [P, S], BF16, kind="ExternalInput").ap()
    mask2 = nc.dram_tensor("mask2", [P, 2 * P], BF16,
                           kind="ExternalInput").ap()
    ident = nc.dram_tensor("ident", [P, P], BF16, kind="ExternalInput").ap()
    out = nc.dram_tensor("out", [B * RSH, D], F32, kind="ExternalOutput").ap()

    with tile.TileContext(nc) as tc:
        with (
            tc.tile_pool(name="const", bufs=1) as const,
            tc.tile_pool(name="dram", bufs=1, space="DRAM") as dram,
            # PSUM plan (8 banks): 2x 1-bank accumulators (qkv passes,
            # vtp, outproj), two 2-bank scores slots (both heads of the
            # pair, double-buffered), two 1-bank o accumulators
            tc.tile_pool(name="psacc", bufs=2, space="PSUM") as psacc,
            tc.tile_pool(name="pssp", bufs=2, space="PSUM") as pssp,
            tc.tile_pool(name="pso", bufs=1, space="PSUM") as pso,
            tc.tile_pool(name="xg", bufs=CH) as xgp,
            tc.tile_pool(name="drain", bufs=3) as drainp,
            tc.tile_pool(name="ptp", bufs=3) as ptp,
            tc.tile_pool(name="nrm", bufs=2) as nrm,
            tc.tile_pool(name="otp", bufs=2) as otp,
            tc.tile_pool(name="wos", bufs=4 * CH) as wosp,
            tc.tile_pool(name="orp", bufs=1) as orp,
            tc.tile_pool(name="outs", bufs=2) as outsp,
        ):
            # ---- constants resident in SBUF ----
            # wT chunk loads are interleaved with the first qkv block's xg
            # chunks (emitted inside the stepped generator below) and the
            # tables load after that block's matmul phase, so the DMA rings
            # deliver the first block's operands first (ramp, not 14us)
            wT_sb = [const.tile([P, E3], BF16, name=f"wT{c}")
                     for c in range(CH)]
            cos_sb = const.tile([P, S], BF16)
            sin_sb = const.tile([P, S], BF16)
            mask2_sb = const.tile([P, 2, P], BF16)
            id_sb = const.tile([P, P], BF16)

            st = {}
            for b in range(B):
                st[b] = {
                    # raw projections, [e, n] layout, 2 q-head pairs + k|v
                    "q2": [const.tile([P, S], BF16, name=f"q2_{b}{i}")
                           for i in range(2)],
                    "kv": const.tile([P, S], BF16, name=f"kv{b}"),
                    # post-rope
                    "qr": [const.tile([P, S], BF16, name=f"qr{b}{i}")
                           for i in range(2)],
                    # k stored twice (partitions 0-63 and 64-127) so the
                    # pair's two heads can run on both PE row tiles
                    "kr": const.tile([P, S], BF16, name=f"kr{b}"),
                    "sw": const.tile([P, S], BF16, name=f"sw{b}"),
                    "vA": const.tile([P, STC * VAW], BF16, name=f"vA{b}"),
                }
                nc.vector.memset(st[b]["vA"], 0.0)
                ones_col = st[b]["vA"].rearrange(
                    "p (t w) -> p t w", w=VAW)[:, :, HD:HD + 1]
                nc.vector.memset(ones_col, 1.0)

            a2a_in = dram.tile([B, HL, NCORES, HD, RSH], BF16)
            a2a_out = dram.tile([B, HL, NCORES, HD, RSH], BF16)

            # orT split per (batch, head pair): tile-granular dependency
            # tracking would otherwise chain every outproj matmul to the
            # last AllToAll
            orT = {(b, hp): orp.tile([P, NCORES * RSH], BF16,
                                     name=f"orT{b}{hp}", tag=f"orT{b}{hp}")
                   for b in range(B) for hp in range(2)}

            def a2a_pair(b, hp):
                """Issue both collectives of a pair, then both orT
                stagings - so the second collective isn't queued behind
                the first's completion wait."""
                for h in (2 * hp, 2 * hp + 1):
                    nc.gpsimd.collective_compute(
                        "AllToAll",
                        mybir.AluOpType.bypass,
                        replica_groups=[list(range(NCORES))],
                        ins=[a2a_in[b, h].opt()],
                        outs=[a2a_out[b, h].opt()],
                    )
                for h in (2 * hp, 2 * hp + 1):
                    k = h % 2
                    for s in range(NCORES):
                        nc.gpsimd.dma_start(
                            out=orT[b, hp][k * HD:(k + 1) * HD,
                                           s * RSH:(s + 1) * RSH],
                            in_=a2a_out[b, h, s, :, :])

            def run(gen):
                for _ in gen:
                    pass

            class Filler:
                def __init__(self, gens, start_delay=0):
                    self.gens = list(gens)
                    self.idx = 0
                    self.delay = start_delay

                def __call__(self, n=1):
                    if self.delay > 0:
                        self.delay -= 1
                        return
                    emitted = 0
                    while emitted < n and self.idx < len(self.gens):
                        try:
                            next(self.gens[self.idx])
                            emitted += 1
                        except StopIteration:
                            self.idx += 1

                def drain(self, upto):
                    """Fully emit generators 0..upto."""
                    while self.idx <= upto and self.idx < len(self.gens):
                        try:
                            next(self.gens[self.idx])
                        except StopIteration:
                            self.idx += 1

            # ---- emission order = engine-queue order = priority ----
            # qkv(0,0) runs plain; later blocks fill attention PE gaps.
            qgens = Filler(
                [_qkv_block(nc, bb, nb, xT, wT_sb, cos_sb, sin_sb, id_sb,
                            st[bb], xgp, psacc, drainp)
                 for bb in range(B) for nb in range(NBB)])
            # ramp: step block (0,0)'s chunk loop manually, issuing wT[c]
            # just ahead of chunk c's matmuls; tables go after the chunk
            # loop (rope/mask/vtp consume them later)
            g0 = qgens.gens[0]
            for c in range(CH):
                nc.gpsimd.dma_start(out=wT_sb[c],
                                    in_=wT[c * P:(c + 1) * P, :])
                next(g0)
            nc.scalar.dma_start(out=cos_sb, in_=cosT)
            nc.scalar.dma_start(out=sin_sb, in_=sinPM)
            nc.scalar.dma_start(out=mask2_sb, in_=mask2)
            nc.scalar.dma_start(out=id_sb, in_=ident)
            qgens.drain(0)  # rest of qkv(0,0)
            for nb in range(NBB):
                _attn_pair(nc, 0, 0, st[0], mask2_sb, a2a_in,
                           pssp, pso, ptp, nrm, otp, j_range=[nb],
                           filler=qgens)
                if nb + 1 < NBB:
                    qgens.drain(nb + 1)  # qkv(0,nb+1) before j=nb+1
            a2a_pair(0, 0)
            _attn_pair(nc, 0, 1, st[0], mask2_sb, a2a_in,
                       pssp, pso, ptp, nrm, otp, filler=qgens)
            qgens.drain(7)  # rest of batch-1 qkv
            # the full wo loads once, on GpSimd, emitted BEFORE the next
            # collectives so no orT wait sits ahead of them; the issues
            # execute during the batch-0 pair-1 span when GpSimd is idle
            wosr = {db: _load_wos(nc, woT, wosp, db, nc.gpsimd)
                    for db in range(4)}
            a2a_pair(0, 1)
            # batch 1 attention; fill with batch-0 output projection.
            # start_delay covers orT[0,1]'s dependency on batch 0's last
            # AllToAll so the first hp1 filler matmul doesn't stall the
            # PE queue mid-pair.
            f_op0 = Filler(
                [_outproj_gen(nc, db, 0, out, orT, wosr[db], psacc, outsp)
                 for db in (0, 1)])
            _attn_pair(nc, 1, 0, st[1], mask2_sb, a2a_in,
                       pssp, pso, ptp, nrm, otp, filler=f_op0)
            f_op0(10000)
            a2a_pair(1, 0)
            f_op1 = Filler(
                [_outproj_gen(nc, db, 0, out, orT, wosr[db], psacc, outsp)
                 for db in (2,)])
            _attn_pair(nc, 1, 1, st[1], mask2_sb, a2a_in,
                       pssp, pso, ptp, nrm, otp, filler=f_op1)
            f_op1(10000)
            a2a_pair(1, 1)
            # batch-0 db3 pass held back as PE work for the final
            # AllToAlls' flight (no collective dependencies)
            run(_outproj_gen(nc, 3, 0, out, orT, wosr[3], psacc, outsp))
            # batch-1 outproj: hp0 waves (need only pair-0 collectives,
            # long done) execute during the final AllToAlls' flight; hp1
            # waves wait on the last collective.  All 8 PSUM banks are
            # free now - allocate every (db, mt) accumulator explicitly
            # so no wave serializes behind another's drain.
            spbA = pssp.tile([P, 2, JW], F32, name="sp", tag="sp")
            spbB = pssp.tile([P, 2, JW], F32, name="sp", tag="sp")
            opb = {
                0: [pso.tile([P, JW], F32, name="opA", tag="oe"),
                    pso.tile([P, JW], F32, name="opB", tag="oo")],
                1: [psacc.tile([P, JW], F32, name="opC", tag="ps"),
                    psacc.tile([P, JW], F32, name="opD", tag="ps")],
                2: [spbA[:, 0, :], spbA[:, 1, :]],
                3: [spbB[:, 0, :], spbB[:, 1, :]],
            }
            # hp -> s -> mt -> db order: 4 consecutive matmuls share the
            # orT stationary slice (rhs varies), maximizing weight reuse
            MT = RSH // P
            for hp in range(2):
                for si, s_ in enumerate(range(NCORES)):
                    for mt in range(MT):
                        lhsT = orT[1, hp][:, s_ * RSH + mt * P:
                                          s_ * RSH + (mt + 1) * P]
                        for db in range(4):
                            nc.tensor.matmul(
                                opb[db][mt][:, :],
                                lhsT=lhsT,
                                rhs=wosr[db][2 * s_ + hp],
                                start=(hp == 0 and si == 0),
                                stop=(hp == 1 and si == NCORES - 1))
            for db in range(4):
                for mt in range(MT):
                    osb = outsp.tile([P, JW], F32, name="osb")
                    nc.vector.tensor_copy(out=osb, in_=opb[db][mt])
                    eng = (nc.sync, nc.scalar)[(2 * db + mt) % 2]
                    eng.dma_start(
                        out=out[RSH + mt * P:RSH + (mt + 1) * P,
                                db * JW:(db + 1) * JW],
                        in_=osb)

    nc.compile()
    return nc


def _qkv_block(nc, b, nb, xT, wT_sb, cos_sb, sin_sb, id_sb, stb, xgp, psacc,
               drainp):
    """Weights-stationary projection for one 512-column n-block, followed by
    its rope, k-duplicate, and v-transpose.  Two passes over the resident
    x chunks (e-tiles {0,1} then {2}) hold only 2 PSUM banks.  Generator:
    yields roughly every two matmuls (~1us PE quanta)."""
    q2, kv = stb["q2"], stb["kv"]
    sw = stb["sw"]
    TPB = NBW // P  # t-chunks per n-block
    vAv = stb["vA"].rearrange("p (t w) -> p t w", w=VAW)[:, :, 0:HD]
    nbg = b * NBB + nb              # global n-block
    n0 = nb * NBW                   # within-batch n
    psA = [psacc.tile([P, NBW], F32, name=f"psA{e}", tag="ps")
           for e in range(2)]
    xgs = []
    for c in range(CH):
        xg = xgp.tile([P, NBW], BF16)
        xgs.append(xg)
        # one contiguous 128KB block per chunk.  For batch 0 alternate
        # issue engines (gpsimd has no collective waits yet).
        if b == 0 and nb == 0:
            eng = (nc.sync, nc.gpsimd, nc.scalar)[c % 3]
        elif b == 0 and c % 2:
            eng = nc.gpsimd
        else:
            eng = nc.sync
        eng.dma_start(out=xg, in_=xT[c, nbg])
        for e in range(2):
            nc.tensor.matmul(
                psA[e][:, :],
                lhsT=wT_sb[c][:, e * P:(e + 1) * P],
                rhs=xg,
                start=(c == 0), stop=(c == CH - 1))
        yield
    for e in range(2):
        nc.vector.tensor_copy(out=q2[e][:, n0:n0 + NBW], in_=psA[e])
    yield
    psB = psacc.tile([P, NBW], F32, name="psB", tag="ps")
    for c in range(CH):
        nc.tensor.matmul(
            psB[:, :],
            lhsT=wT_sb[c][:, 2 * P:3 * P],
            rhs=xgs[c],
            start=(c == 0), stop=(c == CH - 1))
        if c % 2:
            yield
    nc.vector.tensor_copy(out=kv[:, n0:n0 + NBW], in_=psB)
    yield

    # rope this n-block (partition pair-swap via strided SBUF->SBUF DMA)
    for e in range(2):
        _rope_t(nc, drainp, q2[e], stb["qr"][e], sw, cos_sb, sin_sb,
                P, n0)
        yield
    _rope_t(nc, drainp, kv, stb["kr"], sw, cos_sb, sin_sb, HD, n0)
    nc.sync.dma_start(out=stb["kr"][HD:P, n0:n0 + NBW],
                      in_=stb["kr"][0:HD, n0:n0 + NBW])
    yield

    # v: PE transpose to natural [t, hd] + ones column
    vtp = psacc.tile([P, TPB * HD], BF16, name="vtp", tag="ps",
                     padded_shape=[P, 2 * TPB * HD])
    for tl in range(TPB):
        t = nb * TPB + tl
        nc.tensor.transpose(vtp[:, tl * HD:(tl + 1) * HD],
                            kv[HD:P, t * P:(t + 1) * P], id_sb[HD:P, HD:P])
    nc.vector.tensor_copy(
        out=vAv[:, nb * TPB:(nb + 1) * TPB, :],
        in_=vtp.rearrange("p (t w) -> p t w", w=HD))
    yield


def _rope_t(nc, drainp, src, dst, sw, cos_sb, sin_sb, rows, n0):
    """dst[0:rows, n0:n0+NBW] = rope(src[...]) in [e, n] layout.

    Pairs are adjacent partitions; sw is scratch for the pair-swapped copy.
    cos_sb[p, s] = cos(ang[s, p//2 % 32]); sin_sb has the -/+ sign baked in:
    sin_sb[2i] = -sin, sin_sb[2i+1] = +sin."""
    n1 = n0 + NBW
    # sw[2i] = src[2i+1], sw[2i+1] = src[2i]
    nc.sync.dma_start(out=sw[0:rows:2, n0:n1], in_=src[1:rows:2, n0:n1])
    nc.sync.dma_start(out=sw[1:rows:2, n0:n1], in_=src[0:rows:2, n0:n1])
    t1 = drainp.tile([P, NBW], BF16, name="t1", tag="t1", bufs=1)
    t2 = drainp.tile([P, NBW], BF16, name="t2", tag="t2", bufs=1)
    nc.vector.tensor_mul(t1[0:rows], src[0:rows, n0:n1],
                         cos_sb[0:rows, n0:n1])
    nc.vector.tensor_mul(t2[0:rows], sw[0:rows, n0:n1],
                         sin_sb[0:rows, n0:n1])
    nc.vector.tensor_add(dst[0:rows, n0:n1], t1[0:rows], t2[0:rows])


def _attn_pair(nc, b, hp, stb, mask2_sb, a2a_in, pssp, pso, ptp,
               nrm, otp, j_range=None, filler=None, fill_every=1):
    """Causal attention for one (batch, head pair).  The pair's two heads
    run concurrently on the PE's two 64-row tiles during scores; one
    [P, 2, JW] PSUM tile holds both heads' strip so a single exp drains it.
    PV is emitted one strip behind scores (software pipelining) and sp is
    double-buffered, so the PE queue never sits directly behind the exp.
    filler() emits ~1us of foreign PE work per strip to cover the
    exp-bound slack."""
    qr, kr, vA = stb["qr"], stb["kr"], stb["vA"]
    qp = qr[hp]
    for j in (range(JB) if j_range is None else j_range):
        n0 = j * JW
        ni = (n0 + JW) // P
        o_e = pso.tile([P, JW], F32, name="o_e", tag="oe")
        o_o = pso.tile([P, JW], F32, name="o_o", tag="oo")
        prev = None
        for i in range(ni):
            d = max(0, i * P - n0)
            sp = pssp.tile([P, 2, JW], F32, name="sp", tag="sp")
            # even head on row tile (0,0), odd head on (64,0) - concurrent
            nc.tensor.matmul(
                sp[:, 0, d:JW],
                lhsT=kr[0:HD, i * P:(i + 1) * P],
                rhs=qp[0:HD, n0 + d:n0 + JW],
                start=True, stop=True)
            nc.tensor.matmul(
                sp[:, 1, d:JW],
                lhsT=kr[HD:P, i * P:(i + 1) * P],
                rhs=qp[HD:P, n0 + d:n0 + JW],
                start=True, stop=True)
            pt = ptp.tile([P, 2, JW], BF16, name="pt")
            # exp starts at the strip's causal column: cols < d are fully
            # masked and never read downstream
            nc.scalar.activation(out=pt[:, :, d:JW], in_=sp[:, :, d:JW],
                                 func=mybir.ActivationFunctionType.Exp)
            if i * P >= n0:
                # only the [128,128] strip at cols [d, d+128) is partial;
                # both heads' slots share one masked multiply
                nc.vector.tensor_mul(
                    pt[:, :, d:d + P], pt[:, :, d:d + P], mask2_sb)
            if prev is not None:
                _pv(nc, vA, o_e, o_o, prev, n0, ni)
            prev = (i, pt)
            if filler is not None and i % fill_every == 0:
                filler()
        _pv(nc, vA, o_e, o_o, prev, n0, ni)
        _o_drain(nc, b, 2 * hp, o_e, a2a_in, nrm, otp, n0)
        _o_drain(nc, b, 2 * hp + 1, o_o, a2a_in, nrm, otp, n0)


def _pv(nc, vA, o_e, o_o, prev, n0, ni):
    i, pt = prev
    d = max(0, i * P - n0)
    nc.tensor.matmul(
        o_e[:, d:JW],
        lhsT=vA[:, i * VAW:(i + 1) * VAW],
        rhs=pt[:, 0, d:JW],
        start=(i == 0), stop=(i == ni - 1))
    nc.tensor.matmul(
        o_o[:, d:JW],
        lhsT=vA[:, i * VAW:(i + 1) * VAW],
        rhs=pt[:, 1, d:JW],
        start=(i == 0), stop=(i == ni - 1))


def _o_drain(nc, b, h, o_ps, a2a_in, nrm, otp, n0):
    """Normalize one head's o for this n-block and stage it for the
    AllToAll.  1/l on DVE, replicated across the 64 hd partitions by a
    GpSimd partition_broadcast (no PE work), then one DVE multiply reads
    o straight out of PSUM (the other operand is now SBUF)."""
    l_sb = nrm.tile([1, JW], F32, name="l_sb", tag="l")
    nc.vector.tensor_copy(out=l_sb, in_=o_ps[HD:HD + 1, :])
    r = nrm.tile([1, JW], F32, name="r", tag="r")
    nc.vector.reciprocal_approx_fast(out=r, in_=l_sb)
    rb16 = nrm.tile([1, JW], BF16, name="rb16", tag="r16")
    nc.vector.tensor_copy(out=rb16, in_=r)
    rbc = nrm.tile([HD, JW], BF16, name="rbc", tag="rbc")
    nc.gpsimd.partition_broadcast(rbc, rb16)
    ot = otp.tile([HD, JW], BF16, name="ot")
    nc.vector.tensor_mul(ot, o_ps[0:HD, :], rbc)
    for half in range(JW // RSH):
        dest = (n0 + half * RSH) // RSH
        nc.sync.dma_start(
            out=a2a_in[b, h, dest, :, :],
            in_=ot[:, half * RSH:(half + 1) * RSH])


def _load_wos(nc, woT, wosp, db, eng):
    """Stage one column block's wo chunks; returns the 16 tiles."""
    wos = {}
    for c in range(CH):
        w = wosp.tile([P, JW], BF16, name=f"wos{db}_{c}", tag="wos")
        eng.dma_start(out=w, in_=woT[c, db])
        wos[c] = w
    return wos


def _outproj_wave(nc, db, b, hp, out, orT, wos, ops, outsp):
    """One head-pair wave of a (column-block, batch) o @ wo.T pass into the
    caller-provided pair of PSUM accumulators."""
    MT = RSH // P  # 2 row tiles per batch
    for si, s in enumerate(range(NCORES)):
        c = 2 * s + hp
        for mt in range(MT):
            nc.tensor.matmul(
                ops[mt][:, :],
                lhsT=orT[b, hp][:, s * RSH + mt * P:s * RSH + (mt + 1) * P],
                rhs=wos[c],
                start=(hp == 0 and si == 0),
                stop=(hp == 1 and si == NCORES - 1))
    if hp == 1:
        for mt in range(MT):
            osb = outsp.tile([P, JW], F32, name="osb")
            nc.vector.tensor_copy(out=osb, in_=ops[mt])
            eng = (nc.sync, nc.scalar)[(2 * db + mt) % 2]
            eng.dma_start(
                out=out[b * RSH + mt * P:b * RSH + (mt + 1) * P,
                        db * JW:(db + 1) * JW],
                in_=osb)


def _outproj_gen(nc, db, b, out, orT, wos, psacc, outsp):
    """Full (column-block, batch) pass as a generator, hp-major: all the
    pair-0 contributions (whose collective landed long ago) come first,
    so early filler pulls never sit on the pair-1 collective's semaphore.
    Holds both mt accumulators (2 PSUM banks) for the pass."""
    MT = RSH // P
    ops = [psacc.tile([P, JW], F32, name=f"op{mt}", tag="ps")
           for mt in range(MT)]
    for hp in range(2):
        for mt in range(MT):
            for si, s in enumerate(range(NCORES)):
                c = 2 * s + hp
                nc.tensor.matmul(
                    ops[mt][:, :],
                    lhsT=orT[b, hp][:, s * RSH + mt * P:
                                    s * RSH + (mt + 1) * P],
                    rhs=wos[c],
                    start=(hp == 0 and si == 0),
                    stop=(hp == 1 and si == NCORES - 1))
                if si % 2:
                    yield
    for mt in range(MT):
        osb = outsp.tile([P, JW], F32, name="osb")
        nc.vector.tensor_copy(out=osb, in_=ops[mt])
        nc.sync.dma_start(
            out=out[b * RSH + mt * P:b * RSH + (mt + 1) * P,
                    db * JW:(db + 1) * JW],
            in_=osb)
        yield


def _host_prep(x, freqs_cis, wq, wk, wv, wo):
    """Build per-core input maps (numpy only)."""
    x = np.asarray(x, np.float32)
    freqs_cis = np.asarray(freqs_cis, np.float32)
    wq = np.asarray(wq, np.float32)
    wk = np.asarray(wk, np.float32)
    wv = np.asarray(wv, np.float32)
    wo = np.asarray(wo, np.float32)
    bf = ml_dtypes.bfloat16

    # pre-tiled: [c, nb, p, n] with each (c, nb) block contiguous
    xT = np.ascontiguousarray(
        x.reshape(BS, D).T.reshape(CH, P, B * NBB, NBW)
        .transpose(0, 2, 1, 3)).astype(bf)
    woT = np.ascontiguousarray(
        wo.T.reshape(CH, P, D // JW, JW).transpose(0, 2, 1, 3)).astype(bf)
    scale = 1.0 / np.sqrt(np.float32(HD))

    # transposed-layout rope tables: [p, s]
    cos = freqs_cis[:, :, 0]   # [S, 32]
    sin = freqs_cis[:, :, 1]
    pair = (np.arange(P) // 2) % (HD // 2)
    sign = np.where(np.arange(P) % 2 == 0, -1.0, 1.0).astype(np.float32)
    cosT = np.ascontiguousarray(cos[:, pair].T).astype(bf)    # [P, S]
    sinPM = (np.ascontiguousarray(sin[:, pair].T) * sign[:, None]).astype(bf)

    # upper triangle incl diagonal: valid where col >= row; duplicated so
    # one DVE op masks both heads' slots of a [P, 2, P] strip
    maskb = (np.arange(P)[None, :] >= np.arange(P)[:, None]).astype(bf)
    mask2 = np.ascontiguousarray(
        np.stack([maskb, maskb], axis=1).reshape(P, 2 * P))

    identm = np.eye(P, dtype=bf)

    in_maps = []
    for r in range(NCORES):
        wq_r = wq[r * EQ:(r + 1) * EQ] * scale
        wk_r = wk[r * EK:(r + 1) * EK]
        wv_r = wv[r * EV:(r + 1) * EV]
        wTn = np.ascontiguousarray(
            np.concatenate([wq_r.T, wk_r.T, wv_r.T], axis=1)).astype(bf)
        in_maps.append({
            "xT": xT, "wT": wTn, "woT": woT,
            "cosT": cosT, "sinPM": sinPM, "mask2": mask2, "ident": identm,
        })
    return in_maps


def kernel(x, freqs_cis, wq, wk, wv, wo):
    if "nc" not in _CACHE:
        _CACHE["nc"] = _build_nc()
    nc = _CACHE["nc"]

    in_maps = _host_prep(x, freqs_cis, wq, wk, wv, wo)
    trace = bool(int(os.environ.get("KPROF", "0")))
    res = run_bass_kernel_spmd(nc, in_maps, core_ids=list(range(NCORES)),
                               trace=trace)
    if trace:
        _CACHE["last_results"] = res

    full = np.empty((BS, D), np.float32)
    for r in range(NCORES):
        o = res.results[r]["out"]
        full[r * RSH:(r + 1) * RSH] = o[0:RSH]
        full[S + r * RSH:S + (r + 1) * RSH] = o[RSH:2 * RSH]
    return full.reshape(B, S, D)


if __name__ == "__main__":
    rng = np.random.default_rng(0)
    ins = {
        "x": rng.standard_normal((B, S, D), np.float32),
        "freqs_cis": rng.standard_normal((S, HD // 2, 2), np.float32),
        "wq": (rng.standard_normal((H * HD, D)) * 0.02).astype(np.float32),
        "wk": (rng.standard_normal((KV * HD, D)) * 0.02).astype(np.float32),
        "wv": (rng.standard_normal((KV * HD, D)) * 0.02).astype(np.float32),
        "wo": (rng.standard_normal((D, H * HD)) * 0.02).astype(np.float32),
    }
    out = kernel(**ins)
    print("kernel ran, out shape", out.shape, "finite:", np.isfinite(out).all())



# revision 14
# speedup vs baseline: 1.1390x; 1.1390x over previous
"""Trainium2 Bass kernel for nn_Attention_48601849922045.

GQA attention layer (B=2, S=2048, D=2048, H=32 q-heads, KV=8 kv-heads, HD=64)
with llama RoPE, causal softmax, and output projection.

Sharding: tensor-parallel over heads across 8 cores - each core owns one KV
group (1 kv head + its 4 q heads).  x is replicated; per core:

  1. QKV projection, weights-stationary, two passes over the resident x
     chunks (e-tiles {q0,q1} then {k|v}) so it needs only 2 PSUM banks.
  2. RoPE applied in [e, n] layout: partition pair-swap via two strided
     SBUF->SBUF DMAs, then 3 large DVE ops against host-built cos/sin
     tables.  v (no rope) is PE-transposed to natural [t, hd] layout and
     augmented with a ones column so the PV matmul also produces the
     softmax denominator.
  3. Attention per (b, head-PAIR): the local q heads are processed two at
     a time using PE row tiling - kr keeps k duplicated on partitions
     0-63 and 64-127, and qr keeps the pair's heads on the two halves, so
     the even head's scores matmul runs on row tile (0,0) while the odd
     head's runs concurrently on row tile (64,0).  This halves the scores
     phase (K=64 would otherwise idle half the PE array).  Both heads'
     score strips land in one [P, 2, JW] PSUM tile (double-buffered),
     giving a single exp per strip; the exp is sliced to start at the
     strip's causal column, skipping the fully-masked region.  PV is
     K=128 (full array), software-pipelined one strip behind scores.
  4. Eight small per-(b,h) bf16 AllToAlls reshard o from head-sharded to
     row-sharded.  The reshard staging tile orT is SPLIT per head pair:
     a single tile would make every outproj matmul wait on the LAST
     collective (dependencies are tracked per tile), killing the tail.
  5. Row-parallel output projection with the full wo resident in SBUF
     (loaded once), micro-interleaved into the attention phases via
     filler generators (~1us PE quanta) so the PE keeps streaming while
     exp paces the attention pipeline; the final head-pair waves run
     during the last AllToAlls' flight.

Scheduling notes (hard-won): every engine queue executes in order, so a
filler matmul whose inputs aren't ready stalls every attention matmul
behind it (hence start_delay and the orT split); the exp chain on ACT is
the attention pipeline clock, so nothing else may sit on the ACT queue
mid-kernel; both collectives of a pair are issued before either's orT
staging DMAs so the second doesn't queue behind the first's wait.

Host side only shards/transposes inputs and concatenates the 8 output
row-shards.
"""

import os

import numpy as np
import ml_dtypes

import concourse.bass as bass
import concourse.bacc as bacc
import concourse.tile as tile
import concourse.mybir as mybir
from concourse.bass_utils import run_bass_kernel_spmd

P = 128
B, S, D = 2, 2048, 2048
H, KV, HD = 32, 8, 64
NCORES = 8
HL = H // NCORES          # 4 local q heads
BS = B * S                # 4096 rows
EQ, EK, EV = HL * HD, HD, HD
E3 = EQ + EK + EV         # 384 = 3 PE tiles of 128
ET = E3 // P              # 3 e-tiles (0,1: q heads, 2: k|v stacked)
CH = D // P               # 16 contraction chunks
STC = S // P              # 16 t-chunks per batch
NBW = 512                 # qkv n-block width
NBB = S // NBW            # 4 n-blocks per batch
JW = 512                  # attention n-block width
JB = S // JW
RSH = BS // NCORES // B   # 256 rows per (core, batch)
VAW = P                   # v-aug stride: ones at col 64, zero-padded
                          # to 128 cols so the PV stationary gets FWL

F32 = mybir.dt.float32
BF16 = mybir.dt.bfloat16

_CACHE = {}


def _build_nc():
    nc = bacc.Bacc("TRN2", target_bir_lowering=False, debug=False,
                   num_devices=NCORES)

    # xT and woT arrive pre-tiled so every SBUF tile is one contiguous
    # 128KB DRAM block (1KB-row strided loads only reach ~50GB/s)
    xT = nc.dram_tensor("xT", [CH, B * NBB, P, NBW], BF16,
                        kind="ExternalInput").ap()
    wT = nc.dram_tensor("wT", [D, E3], BF16, kind="ExternalInput").ap()
    woT = nc.dram_tensor("woT", [CH, D // JW, P, JW], BF16,
                         kind="ExternalInput").ap()
    cosT = nc.dram_tensor("cosT", [P, S], BF16, kind="ExternalInput").ap()
    sinPM = nc.dram_tensor("sinPM", [P, S], BF16, kind="ExternalInput").ap()
    mask2 = nc.dram_tensor("mask2", [P, 2 * P], BF16,
                           kind="ExternalInput").ap()
    ident = nc.dram_tensor("ident", [P, P], BF16, kind="ExternalInput").ap()
    out = nc.dram_tensor("out", [B * RSH, D], F32, kind="ExternalOutput").ap()

    with tile.TileContext(nc) as tc:
        with (
            tc.tile_pool(name="const", bufs=1) as const,
            tc.tile_pool(name="dram", bufs=1, space="DRAM") as dram,
            # PSUM plan (8 banks): 2x 1-bank accumulators (qkv passes,
            # vtp, outproj), two 2-bank scores slots (both heads of the
            # pair, double-buffered), two 1-bank o accumulators
            tc.tile_pool(name="psacc", bufs=2, space="PSUM") as psacc,
            tc.tile_pool(name="pssp", bufs=2, space="PSUM") as pssp,
            tc.tile_pool(name="pso", bufs=1, space="PSUM") as pso,
            tc.tile_pool(name="xg", bufs=CH) as xgp,
            tc.tile_pool(name="drain", bufs=3) as drainp,
            tc.tile_pool(name="ptp", bufs=3) as ptp,
            tc.tile_pool(name="nrm", bufs=2) as nrm,
            tc.tile_pool(name="otp", bufs=2) as otp,
            tc.tile_pool(name="wos", bufs=4 * CH) as wosp,
            tc.tile_pool(name="orp", bufs=1) as orp,
            tc.tile_pool(name="outs", bufs=2) as outsp,
        ):
            # ---- constants resident in SBUF ----
            # wT chunk loads are interleaved with the first qkv block's xg
            # chunks (emitted inside the stepped generator below) and the
            # tables load after that block's matmul phase, so the DMA rings
            # deliver the first block's operands first (ramp, not 14us)
            wT_sb = [const.tile([P, E3], BF16, name=f"wT{c}")
                     for c in range(CH)]
            cos_sb = const.tile([P, S], BF16)
            sin_sb = const.tile([P, S], BF16)
            mask2_sb = const.tile([P, 2, P], BF16)
            id_sb = const.tile([P, P], BF16)
            ones_sb = const.tile([1, HD], BF16)
            nc.vector.memset(ones_sb, 1.0)

            st = {}
            for b in range(B):
                st[b] = {
                    # raw projections, [e, n] layout, 2 q-head pairs + k|v
                    "q2": [const.tile([P, S], BF16, name=f"q2_{b}{i}")
                           for i in range(2)],
                    "kv": const.tile([P, S], BF16, name=f"kv{b}"),
                    # post-rope
                    "qr": [const.tile([P, S], BF16, name=f"qr{b}{i}")
                           for i in range(2)],
                    # k stored twice (partitions 0-63 and 64-127) so the
                    # pair's two heads can run on both PE row tiles
                    "kr": const.tile([P, S], BF16, name=f"kr{b}"),
                    "sw": const.tile([P, S], BF16, name=f"sw{b}"),
                    "vA": const.tile([P, STC * VAW], BF16, name=f"vA{b}"),
                }
                nc.vector.memset(st[b]["vA"], 0.0)
                ones_col = st[b]["vA"].rearrange(
                    "p (t w) -> p t w", w=VAW)[:, :, HD:HD + 1]
                nc.vector.memset(ones_col, 1.0)

            a2a_in = dram.tile([B, HL, NCORES, HD, RSH], BF16)
            a2a_out = dram.tile([B, HL, NCORES, HD, RSH], BF16)

            # orT split per (batch, head pair): tile-granular dependency
            # tracking would otherwise chain every outproj matmul to the
            # last AllToAll
            orT = {(b, hp): orp.tile([P, NCORES * RSH], BF16,
                                     name=f"orT{b}{hp}", tag=f"orT{b}{hp}")
                   for b in range(B) for hp in range(2)}

            def a2a_pair(b, hp):
                """Issue both collectives of a pair, then both orT
                stagings - so the second collective isn't queued behind
                the first's completion wait."""
                for h in (2 * hp, 2 * hp + 1):
                    nc.gpsimd.collective_compute(
                        "AllToAll",
                        mybir.AluOpType.bypass,
                        replica_groups=[list(range(NCORES))],
                        ins=[a2a_in[b, h].opt()],
                        outs=[a2a_out[b, h].opt()],
                    )
                for h in (2 * hp, 2 * hp + 1):
                    k = h % 2
                    for s in range(NCORES):
                        nc.gpsimd.dma_start(
                            out=orT[b, hp][k * HD:(k + 1) * HD,
                                           s * RSH:(s + 1) * RSH],
                            in_=a2a_out[b, h, s, :, :])

            def run(gen):
                for _ in gen:
                    pass

            class Filler:
                def __init__(self, gens, start_delay=0):
                    self.gens = list(gens)
                    self.idx = 0
                    self.delay = start_delay

                def __call__(self, n=1):
                    if self.delay > 0:
                        self.delay -= 1
                        return
                    emitted = 0
                    while emitted < n and self.idx < len(self.gens):
                        try:
                            next(self.gens[self.idx])
                            emitted += 1
                        except StopIteration:
                            self.idx += 1

                def drain(self, upto):
                    """Fully emit generators 0..upto."""
                    while self.idx <= upto and self.idx < len(self.gens):
                        try:
                            next(self.gens[self.idx])
                        except StopIteration:
                            self.idx += 1

            # ---- emission order = engine-queue order = priority ----
            # qkv(0,0) runs plain; later blocks fill attention PE gaps.
            qgens = Filler(
                [_qkv_block(nc, bb, nb, xT, wT_sb, cos_sb, sin_sb, id_sb,
                            st[bb], xgp, psacc, drainp)
                 for bb in range(B) for nb in range(NBB)])
            # ramp: step block (0,0)'s chunk loop manually, issuing wT[c]
            # just ahead of chunk c's matmuls; tables go after the chunk
            # loop (rope/mask/vtp consume them later)
            g0 = qgens.gens[0]
            for c in range(CH):
                nc.gpsimd.dma_start(out=wT_sb[c],
                                    in_=wT[c * P:(c + 1) * P, :])
                next(g0)
            nc.scalar.dma_start(out=cos_sb, in_=cosT)
            nc.scalar.dma_start(out=sin_sb, in_=sinPM)
            nc.scalar.dma_start(out=mask2_sb, in_=mask2)
            nc.scalar.dma_start(out=id_sb, in_=ident)
            qgens.drain(0)  # rest of qkv(0,0)
            for nb in range(NBB):
                _attn_pair(nc, 0, 0, st[0], mask2_sb, ones_sb, a2a_in,
                           pssp, pso, ptp, nrm, otp, j_range=[nb],
                           filler=qgens)
                if nb + 1 < NBB:
                    qgens.drain(nb + 1)  # qkv(0,nb+1) before j=nb+1
            a2a_pair(0, 0)
            _attn_pair(nc, 0, 1, st[0], mask2_sb, ones_sb, a2a_in,
                       pssp, pso, ptp, nrm, otp, filler=qgens)
            qgens.drain(7)  # rest of batch-1 qkv
            # the full wo loads once, on GpSimd, emitted BEFORE the next
            # collectives so no orT wait sits ahead of them; the issues
            # execute during the batch-0 pair-1 span when GpSimd is idle
            wosr = {db: _load_wos(nc, woT, wosp, db, nc.gpsimd)
                    for db in range(4)}
            a2a_pair(0, 1)
            # batch 1 attention; fill with batch-0 output projection.
            # start_delay covers orT[0,1]'s dependency on batch 0's last
            # AllToAll so the first hp1 filler matmul doesn't stall the
            # PE queue mid-pair.
            f_op0 = Filler(
                [_outproj_gen(nc, db, 0, out, orT, wosr[db], psacc, outsp)
                 for db in (0, 1)])
            _attn_pair(nc, 1, 0, st[1], mask2_sb, ones_sb, a2a_in,
                       pssp, pso, ptp, nrm, otp, filler=f_op0)
            f_op0(10000)
            a2a_pair(1, 0)
            f_op1 = Filler(
                [_outproj_gen(nc, db, 0, out, orT, wosr[db], psacc, outsp)
                 for db in (2,)])
            _attn_pair(nc, 1, 1, st[1], mask2_sb, ones_sb, a2a_in,
                       pssp, pso, ptp, nrm, otp, filler=f_op1)
            f_op1(10000)
            a2a_pair(1, 1)
            # batch-0 db3 pass held back as PE work for the final
            # AllToAlls' flight (no collective dependencies)
            run(_outproj_gen(nc, 3, 0, out, orT, wosr[3], psacc, outsp))
            # batch-1 outproj: hp0 waves (need only pair-0 collectives,
            # long done) execute during the final AllToAlls' flight; hp1
            # waves wait on the last collective.  All 8 PSUM banks are
            # free now - allocate every (db, mt) accumulator explicitly
            # so no wave serializes behind another's drain.
            spbA = pssp.tile([P, 2, JW], F32, name="sp", tag="sp")
            spbB = pssp.tile([P, 2, JW], F32, name="sp", tag="sp")
            opb = {
                0: [pso.tile([P, JW], F32, name="opA", tag="oe"),
                    pso.tile([P, JW], F32, name="opB", tag="oo")],
                1: [psacc.tile([P, JW], F32, name="opC", tag="ps"),
                    psacc.tile([P, JW], F32, name="opD", tag="ps")],
                2: [spbA[:, 0, :], spbA[:, 1, :]],
                3: [spbB[:, 0, :], spbB[:, 1, :]],
            }
            # hp -> s -> mt -> db order: 4 consecutive matmuls share the
            # orT stationary slice (rhs varies), maximizing weight reuse
            MT = RSH // P
            for hp in range(2):
                for si, s_ in enumerate(range(NCORES)):
                    for mt in range(MT):
                        lhsT = orT[1, hp][:, s_ * RSH + mt * P:
                                          s_ * RSH + (mt + 1) * P]
                        for db in range(4):
                            nc.tensor.matmul(
                                opb[db][mt][:, :],
                                lhsT=lhsT,
                                rhs=wosr[db][2 * s_ + hp],
                                start=(hp == 0 and si == 0),
                                stop=(hp == 1 and si == NCORES - 1))
            for db in range(4):
                for mt in range(MT):
                    osb = outsp.tile([P, JW], F32, name="osb")
                    nc.vector.tensor_copy(out=osb, in_=opb[db][mt])
                    eng = (nc.sync, nc.scalar)[(2 * db + mt) % 2]
                    eng.dma_start(
                        out=out[RSH + mt * P:RSH + (mt + 1) * P,
                                db * JW:(db + 1) * JW],
                        in_=osb)

    nc.compile()
    return nc


def _qkv_block(nc, b, nb, xT, wT_sb, cos_sb, sin_sb, id_sb, stb, xgp, psacc,
               drainp):
    """Weights-stationary projection for one 512-column n-block, followed by
    its rope, k-duplicate, and v-transpose.  Two passes over the resident
    x chunks (e-tiles {0,1} then {2}) hold only 2 PSUM banks.  Generator:
    yields roughly every two matmuls (~1us PE quanta)."""
    q2, kv = stb["q2"], stb["kv"]
    sw = stb["sw"]
    TPB = NBW // P  # t-chunks per n-block
    vAv = stb["vA"].rearrange("p (t w) -> p t w", w=VAW)[:, :, 0:HD]
    nbg = b * NBB + nb              # global n-block
    n0 = nb * NBW                   # within-batch n
    psA = [psacc.tile([P, NBW], F32, name=f"psA{e}", tag="ps")
           for e in range(2)]
    xgs = []
    for c in range(CH):
        xg = xgp.tile([P, NBW], BF16)
        xgs.append(xg)
        # one contiguous 128KB block per chunk.  For batch 0 alternate
        # issue engines (gpsimd has no collective waits yet).
        if b == 0 and nb == 0:
            eng = (nc.sync, nc.gpsimd, nc.scalar)[c % 3]
        elif b == 0 and c % 2:
            eng = nc.gpsimd
        else:
            eng = nc.sync
        eng.dma_start(out=xg, in_=xT[c, nbg])
        for e in range(2):
            nc.tensor.matmul(
                psA[e][:, :],
                lhsT=wT_sb[c][:, e * P:(e + 1) * P],
                rhs=xg,
                start=(c == 0), stop=(c == CH - 1))
        yield
    for e in range(2):
        nc.vector.tensor_copy(out=q2[e][:, n0:n0 + NBW], in_=psA[e])
    yield
    psB = psacc.tile([P, NBW], F32, name="psB", tag="ps")
    for c in range(CH):
        nc.tensor.matmul(
            psB[:, :],
            lhsT=wT_sb[c][:, 2 * P:3 * P],
            rhs=xgs[c],
            start=(c == 0), stop=(c == CH - 1))
        if c % 2:
            yield
    nc.vector.tensor_copy(out=kv[:, n0:n0 + NBW], in_=psB)
    yield

    # rope this n-block (partition pair-swap via strided SBUF->SBUF DMA)
    for e in range(2):
        _rope_t(nc, drainp, q2[e], stb["qr"][e], sw, cos_sb, sin_sb,
                P, n0)
        yield
    _rope_t(nc, drainp, kv, stb["kr"], sw, cos_sb, sin_sb, HD, n0)
    nc.sync.dma_start(out=stb["kr"][HD:P, n0:n0 + NBW],
                      in_=stb["kr"][0:HD, n0:n0 + NBW])
    yield

    # v: PE transpose to natural [t, hd] + ones column
    vtp = psacc.tile([P, TPB * HD], BF16, name="vtp", tag="ps",
                     padded_shape=[P, 2 * TPB * HD])
    for tl in range(TPB):
        t = nb * TPB + tl
        nc.tensor.transpose(vtp[:, tl * HD:(tl + 1) * HD],
                            kv[HD:P, t * P:(t + 1) * P], id_sb[HD:P, HD:P])
    nc.vector.tensor_copy(
        out=vAv[:, nb * TPB:(nb + 1) * TPB, :],
        in_=vtp.rearrange("p (t w) -> p t w", w=HD))
    yield


def _rope_t(nc, drainp, src, dst, sw, cos_sb, sin_sb, rows, n0):
    """dst[0:rows, n0:n0+NBW] = rope(src[...]) in [e, n] layout.

    Pairs are adjacent partitions; sw is scratch for the pair-swapped copy.
    cos_sb[p, s] = cos(ang[s, p//2 % 32]); sin_sb has the -/+ sign baked in:
    sin_sb[2i] = -sin, sin_sb[2i+1] = +sin."""
    n1 = n0 + NBW
    # sw[2i] = src[2i+1], sw[2i+1] = src[2i]
    nc.sync.dma_start(out=sw[0:rows:2, n0:n1], in_=src[1:rows:2, n0:n1])
    nc.sync.dma_start(out=sw[1:rows:2, n0:n1], in_=src[0:rows:2, n0:n1])
    t1 = drainp.tile([P, NBW], BF16, name="t1", tag="t1", bufs=1)
    t2 = drainp.tile([P, NBW], BF16, name="t2", tag="t2", bufs=1)
    nc.vector.tensor_mul(t1[0:rows], src[0:rows, n0:n1],
                         cos_sb[0:rows, n0:n1])
    nc.vector.tensor_mul(t2[0:rows], sw[0:rows, n0:n1],
                         sin_sb[0:rows, n0:n1])
    nc.vector.tensor_add(dst[0:rows, n0:n1], t1[0:rows], t2[0:rows])


def _attn_pair(nc, b, hp, stb, mask2_sb, ones_sb, a2a_in, pssp, pso, ptp,
               nrm, otp, j_range=None, filler=None, fill_every=1):
    """Causal attention for one (batch, head pair).  The pair's two heads
    run concurrently on the PE's two 64-row tiles during scores; one
    [P, 2, JW] PSUM tile holds both heads' strip so a single exp drains it.
    PV is emitted one strip behind scores (software pipelining) and sp is
    double-buffered, so the PE queue never sits directly behind the exp.
    filler() emits ~1us of foreign PE work per strip to cover the
    exp-bound slack."""
    qr, kr, vA = stb["qr"], stb["kr"], stb["vA"]
    qp = qr[hp]
    for j in (range(JB) if j_range is None else j_range):
        n0 = j * JW
        ni = (n0 + JW) // P
        o_e = pso.tile([P, JW], F32, name="o_e", tag="oe")
        o_o = pso.tile([P, JW], F32, name="o_o", tag="oo")
        prev = None
        for i in range(ni):
            d = max(0, i * P - n0)
            sp = pssp.tile([P, 2, JW], F32, name="sp", tag="sp")
            # even head on row tile (0,0), odd head on (64,0) - concurrent
            nc.tensor.matmul(
                sp[:, 0, d:JW],
                lhsT=kr[0:HD, i * P:(i + 1) * P],
                rhs=qp[0:HD, n0 + d:n0 + JW],
                start=True, stop=True)
            nc.tensor.matmul(
                sp[:, 1, d:JW],
                lhsT=kr[HD:P, i * P:(i + 1) * P],
                rhs=qp[HD:P, n0 + d:n0 + JW],
                start=True, stop=True)
            pt = ptp.tile([P, 2, JW], BF16, name="pt")
            # exp starts at the strip's causal column: cols < d are fully
            # masked and never read downstream
            nc.scalar.activation(out=pt[:, :, d:JW], in_=sp[:, :, d:JW],
                                 func=mybir.ActivationFunctionType.Exp)
            if i * P >= n0:
                # only the [128,128] strip at cols [d, d+128) is partial;
                # both heads' slots share one masked multiply
                nc.vector.tensor_mul(
                    pt[:, :, d:d + P], pt[:, :, d:d + P], mask2_sb)
            if prev is not None:
                _pv(nc, vA, o_e, o_o, prev, n0, ni)
            prev = (i, pt)
            if filler is not None and i % fill_every == 0:
                filler()
        _pv(nc, vA, o_e, o_o, prev, n0, ni)
        _o_drain(nc, b, 2 * hp, o_e, ones_sb, a2a_in, nrm, otp, n0)
        _o_drain(nc, b, 2 * hp + 1, o_o, ones_sb, a2a_in, nrm, otp, n0)


def _pv(nc, vA, o_e, o_o, prev, n0, ni):
    i, pt = prev
    d = max(0, i * P - n0)
    nc.tensor.matmul(
        o_e[:, d:JW],
        lhsT=vA[:, i * VAW:(i + 1) * VAW],
        rhs=pt[:, 0, d:JW],
        start=(i == 0), stop=(i == ni - 1))
    nc.tensor.matmul(
        o_o[:, d:JW],
        lhsT=vA[:, i * VAW:(i + 1) * VAW],
        rhs=pt[:, 1, d:JW],
        start=(i == 0), stop=(i == ni - 1))


def _o_drain(nc, b, h, o_ps, ones_sb, a2a_in, nrm, otp, n0):
    """Normalize one head's o for this n-block and stage it for the
    AllToAll.  Avoids GpSimd (its queue must stay free to block on
    collective waits) and ACT (the exp chain): 1/l on DVE, then a K=1
    ones-matmul broadcasts r into the unused rows 64..127 of the o bank."""
    l_sb = nrm.tile([1, JW], F32, name="l_sb", tag="l")
    nc.vector.tensor_copy(out=l_sb, in_=o_ps[HD:HD + 1, :])
    r = nrm.tile([1, JW], F32, name="r", tag="r")
    nc.vector.reciprocal_approx_fast(out=r, in_=l_sb)
    rb16 = nrm.tile([1, JW], BF16, name="rb16", tag="r16")
    nc.vector.tensor_copy(out=rb16, in_=r)
    nc.tensor.matmul(o_ps[HD:HD + HD, :], lhsT=ones_sb, rhs=rb16,
                     start=True, stop=True)
    # DVE reads at most one PSUM operand: stage o in SBUF first
    o_f = otp.tile([HD, JW], F32, name="o_f", tag="o_f", bufs=1)
    nc.vector.tensor_copy(out=o_f, in_=o_ps[0:HD, :])
    ot = otp.tile([HD, JW], BF16, name="ot")
    nc.vector.tensor_mul(ot, o_f, o_ps[HD:HD + HD, :])
    for half in range(JW // RSH):
        dest = (n0 + half * RSH) // RSH
        nc.sync.dma_start(
            out=a2a_in[b, h, dest, :, :],
            in_=ot[:, half * RSH:(half + 1) * RSH])


def _load_wos(nc, woT, wosp, db, eng):
    """Stage one column block's wo chunks; returns the 16 tiles."""
    wos = {}
    for c in range(CH):
        w = wosp.tile([P, JW], BF16, name=f"wos{db}_{c}", tag="wos")
        eng.dma_start(out=w, in_=woT[c, db])
        wos[c] = w
    return wos


def _outproj_wave(nc, db, b, hp, out, orT, wos, ops, outsp):
    """One head-pair wave of a (column-block, batch) o @ wo.T pass into the
    caller-provided pair of PSUM accumulators."""
    MT = RSH // P  # 2 row tiles per batch
    for si, s in enumerate(range(NCORES)):
        c = 2 * s + hp
        for mt in range(MT):
            nc.tensor.matmul(
                ops[mt][:, :],
                lhsT=orT[b, hp][:, s * RSH + mt * P:s * RSH + (mt + 1) * P],
                rhs=wos[c],
                start=(hp == 0 and si == 0),
                stop=(hp == 1 and si == NCORES - 1))
    if hp == 1:
        for mt in range(MT):
            osb = outsp.tile([P, JW], F32, name="osb")
            nc.vector.tensor_copy(out=osb, in_=ops[mt])
            eng = (nc.sync, nc.scalar)[(2 * db + mt) % 2]
            eng.dma_start(
                out=out[b * RSH + mt * P:b * RSH + (mt + 1) * P,
                        db * JW:(db + 1) * JW],
                in_=osb)


def _outproj_gen(nc, db, b, out, orT, wos, psacc, outsp):
    """Full (column-block, batch) pass as a generator, hp-major: all the
    pair-0 contributions (whose collective landed long ago) come first,
    so early filler pulls never sit on the pair-1 collective's semaphore.
    Holds both mt accumulators (2 PSUM banks) for the pass."""
    MT = RSH // P
    ops = [psacc.tile([P, JW], F32, name=f"op{mt}", tag="ps")
           for mt in range(MT)]
    for hp in range(2):
        for mt in range(MT):
            for si, s in enumerate(range(NCORES)):
                c = 2 * s + hp
                nc.tensor.matmul(
                    ops[mt][:, :],
                    lhsT=orT[b, hp][:, s * RSH + mt * P:
                                    s * RSH + (mt + 1) * P],
                    rhs=wos[c],
                    start=(hp == 0 and si == 0),
                    stop=(hp == 1 and si == NCORES - 1))
                if si % 2:
                    yield
    for mt in range(MT):
        osb = outsp.tile([P, JW], F32, name="osb")
        nc.vector.tensor_copy(out=osb, in_=ops[mt])
        nc.sync.dma_start(
            out=out[b * RSH + mt * P:b * RSH + (mt + 1) * P,
                    db * JW:(db + 1) * JW],
            in_=osb)
        yield


def _host_prep(x, freqs_cis, wq, wk, wv, wo):
    """Build per-core input maps (numpy only)."""
    x = np.asarray(x, np.float32)
    freqs_cis = np.asarray(freqs_cis, np.float32)
    wq = np.asarray(wq, np.float32)
    wk = np.asarray(wk, np.float32)
    wv = np.asarray(wv, np.float32)
    wo = np.asarray(wo, np.float32)
    bf = ml_dtypes.bfloat16

    # pre-tiled: [c, nb, p, n] with each (c, nb) block contiguous
    xT = np.ascontiguousarray(
        x.reshape(BS, D).T.reshape(CH, P, B * NBB, NBW)
        .transpose(0, 2, 1, 3)).astype(bf)
    woT = np.ascontiguousarray(
        wo.T.reshape(CH, P, D // JW, JW).transpose(0, 2, 1, 3)).astype(bf)
    scale = 1.0 / np.sqrt(np.float32(HD))

    # transposed-layout rope tables: [p, s]
    cos = freqs_cis[:, :, 0]   # [S, 32]
    sin = freqs_cis[:, :, 1]
    pair = (np.arange(P) // 2) % (HD // 2)
    sign = np.where(np.arange(P) % 2 == 0, -1.0, 1.0).astype(np.float32)
    cosT = np.ascontiguousarray(cos[:, pair].T).astype(bf)    # [P, S]
    sinPM = (np.ascontiguousarray(sin[:, pair].T) * sign[:, None]).astype(bf)

    # upper triangle incl diagonal: valid where col >= row; duplicated so
    # one DVE op masks both heads' slots of a [P, 2, P] strip
    maskb = (np.arange(P)[None, :] >= np.arange(P)[:, None]).astype(bf)
    mask2 = np.ascontiguousarray(
        np.stack([maskb, maskb], axis=1).reshape(P, 2 * P))

    identm = np.eye(P, dtype=bf)

    in_maps = []
    for r in range(NCORES):
        wq_r = wq[r * EQ:(r + 1) * EQ] * scale
        wk_r = wk[r * EK:(r + 1) * EK]
        wv_r = wv[r * EV:(r + 1) * EV]
        wTn = np.ascontiguousarray(
            np.concatenate([wq_r.T, wk_r.T, wv_r.T], axis=1)).astype(bf)
        in_maps.append({
            "xT": xT, "wT": wTn, "woT": woT,
            "cosT": cosT, "sinPM": sinPM, "mask2": mask2, "ident": identm,
        })
    return in_maps


def kernel(x, freqs_cis, wq, wk, wv, wo):
    if "nc" not in _CACHE:
        _CACHE["nc"] = _build_nc()
    nc = _CACHE["nc"]

    in_maps = _host_prep(x, freqs_cis, wq, wk, wv, wo)
    trace = bool(int(os.environ.get("KPROF", "0")))
    res = run_bass_kernel_spmd(nc, in_maps, core_ids=list(range(NCORES)),
                               trace=trace)
    if trace:
        _CACHE["last_results"] = res

    full = np.empty((BS, D), np.float32)
    for r in range(NCORES):
        o = res.results[r]["out"]
        full[r * RSH:(r + 1) * RSH] = o[0:RSH]
        full[S + r * RSH:S + (r + 1) * RSH] = o[RSH:2 * RSH]
    return full.reshape(B, S, D)


if __name__ == "__main__":
    rng = np.random.default_rng(0)
    ins = {
        "x": rng.standard_normal((B, S, D), np.float32),
        "freqs_cis": rng.standard_normal((S, HD // 2, 2), np.float32),
        "wq": (rng.standard_normal((H * HD, D)) * 0.02).astype(np.float32),
        "wk": (rng.standard_normal((KV * HD, D)) * 0.02).astype(np.float32),
        "wv": (rng.standard_normal((KV * HD, D)) * 0.02).astype(np.float32),
        "wo": (rng.standard_normal((D, H * HD)) * 0.02).astype(np.float32),
    }
    out = kernel(**ins)
    print("kernel ran, out shape", out.shape, "finite:", np.isfinite(out).all())

